# revision 10
# baseline (speedup 1.0000x reference)
"""Trainium2 Bass kernel for a CAM (channel-attention) module.

Reference computation (per batch b):
    v    = x[b].reshape(C, H*W)                  # C x N
    e    = v @ v.T                               # C x C Gram matrix
    attn = softmax(rowmax(e) - e, axis=-1)       # == exp(rowmin(e)-e) / rowsum
    out  = gamma * (attn @ v) + x[b]

Sharding: data-parallel over batch B=16 across 8 NeuronCores (2 batches/core,
no cross-core communication).

Per core, per batch (software-pipelined across batches):
  - x[b] streamed in fp32 quarter-tiles (residual source, kept exact)
  - DVE casts to bf16 quarter copies (matmul operands)
  - PE-transpose bf16 128x128 blocks -> resident vT [128n, kt, 512c] (bf16),
    two kt per PSUM bank, single ACT eviction per pair
  - energy: upper-triangular blocks only (Gram symmetry); lower blocks
    reconstructed by PE-transposing finished rows; m-outer accumulation so
    softmax overlaps the next block's matmuls
  - softmax: rowmin (DVE) + exp with accum_out rowsum (ACT); U unnormalized
  - PE-transpose U -> UT (bf16); raw[c,n] = sum_d U[c,d] v[d,n] (bf16 MMs)
  - eviction fuses normalization+gamma+residual: out = raw*(gamma/Z) + x_fp32
    (so for gamma == 0 the output is bit-exact x)
  - next batch's DMAs + casts are emitted before this batch's second matmul
    so the tensor engine crosses batch boundaries without draining
"""

import numpy as np

P = 128
C = 512
N = 4096
CT = C // P      # 4 c-tiles
NT = N // P      # 32 n-tiles
NP = NT // 2     # 16 transpose pairs
CH = 512         # matmul free-dim chunk
NCH = N // CH    # 8 n-chunks
QN = N // 4      # 1024 quarter width
B = 16
NCORES = 8
BPC = B // NCORES  # batches per core

_CACHE = {}


def _build_program():
    import concourse.bacc as bacc
    import concourse.mybir as mybir
    import concourse.tile as tile
    from concourse.masks import make_identity

    f32 = mybir.dt.float32
    bf16 = mybir.dt.bfloat16
    Alu = mybir.AluOpType
    Act = mybir.ActivationFunctionType

    nc = bacc.Bacc("TRN2", target_bir_lowering=False, debug=False)
    x_d = nc.dram_tensor("x", [BPC, C, N], f32, kind="ExternalInput").ap()
    g_d = nc.dram_tensor("gamma", [1], f32, kind="ExternalInput").ap()
    o_d = nc.dram_tensor("out", [BPC, C, N], f32, kind="ExternalOutput").ap()

    with tile.TileContext(nc) as tc:
        with (
            tc.tile_pool(name="const", bufs=1) as const_pool,
            tc.tile_pool(name="vp", bufs=1) as v_pool,
            tc.tile_pool(name="vhp", bufs=1) as vh_pool,
            tc.tile_pool(name="vtp", bufs=1) as vt_pool,
            tc.tile_pool(name="up", bufs=2) as u_pool,
            tc.tile_pool(name="stat", bufs=2) as st_pool,
            tc.tile_pool(name="outp", bufs=3) as out_pool,
            tc.tile_pool(name="pse", bufs=2, space="PSUM") as ps_e_pool,
            tc.tile_pool(name="pso", bufs=3, space="PSUM") as ps_o_pool,
            tc.tile_pool(name="pstp", bufs=3, space="PSUM") as ps_tp,
        ):
            ident_h = const_pool.tile([P, P], bf16, tag="identh")
            make_identity(nc, ident_h)
            ident_f = const_pool.tile([P, P], f32, tag="identf")
            make_identity(nc, ident_f)
            gamma_bc = const_pool.tile([P, 1], f32, tag="gamma")
            nc.sync.dma_start(gamma_bc, g_d.to_broadcast((P, 1)))

            def emit_load(b, cast_on_act):
                """fp32 quarter loads + bf16 quarter casts, n-interleaved.

                Batch 0 casts on DVE (idle at startup); later batches cast on
                ACT so they never delay the previous batch's mm2 evictions on
                DVE."""
                v_sb = [[None] * 4 for _ in range(CT)]
                v_h = [[None] * 4 for _ in range(CT)]
                for q in range(4):
                    for ct in range(CT):
                        t = v_pool.tile([P, QN], f32, tag=f"v{ct}q{q}",
                                        bufs=2 if q == 0 else 1,
                                        name=f"v_{b}_{ct}_{q}")
                        nc.sync.dma_start(
                            t,
                            x_d[b, ct * P:(ct + 1) * P, q * QN:(q + 1) * QN])
                        v_sb[ct][q] = t
                    for ct in range(CT):
                        th = vh_pool.tile([P, QN], bf16, tag=f"vh{ct}q{q}",
                                          bufs=2 if q == 0 else 1,
                                          name=f"vh_{b}_{ct}_{q}")
                        if cast_on_act:
                            nc.scalar.activation(th, v_sb[ct][q], Act.Copy)
                        else:
                            nc.vector.tensor_copy(th, v_sb[ct][q])
                        v_h[ct][q] = th
                return v_sb, v_h

            state = {}

            def emit_transposes(b, v_h):
                vT = []
                for p in range(NP):
                    ps_t = ps_tp.tile([P, 2, C], bf16, tag="tp",
                                      name=f"ps_tv_{b}_{p}")
                    for sub in range(2):
                        kt = 2 * p + sub
                        q, lk = kt // 8, kt % 8
                        for ct in range(CT):
                            nc.tensor.transpose(
                                ps_t[:, sub, ct * P:(ct + 1) * P],
                                v_h[ct][q][:, lk * P:(lk + 1) * P],
                                ident_h)
                    vt_t = vt_pool.tile([P, 2, C], bf16, tag=f"vt{p}",
                                        name=f"vT_{b}_{p}")
                    nc.scalar.activation(vt_t, ps_t, Act.Copy)
                    vT.append(vt_t)
                return vT

            def emit_softmax(b, m, e_sb, mins, zsum, gz, u_sb):
                nc.vector.tensor_reduce(
                    mins[:, m:m + 1], e_sb[:, m, :],
                    axis=mybir.AxisListType.X, op=Alu.min)
                nc.scalar.activation(
                    u_sb[:, m, :], e_sb[:, m, :], Act.Exp,
                    bias=mins[:, m:m + 1], scale=-1.0,
                    accum_out=zsum[:, m:m + 1])
                nc.vector.reciprocal(gz[:, m:m + 1], zsum[:, m:m + 1])
                nc.vector.tensor_tensor(
                    gz[:, m:m + 1], gz[:, m:m + 1], gamma_bc, Alu.mult)

            def emit_lower_blocks(b, m, e_sb):
                """e[m-rows, j<m cols] = transpose(e[j-rows, m-cols])."""
                ps_x = ps_tp.tile([P, CH], f32, tag="tp", name=f"ps_ex_{b}_{m}")
                for j in range(m):
                    nc.tensor.transpose(
                        ps_x[:, j * P:(j + 1) * P],
                        e_sb[:, j, m * P:(m + 1) * P],
                        ident_f)
                nc.scalar.activation(e_sb[:, m, :m * P], ps_x[:, :m * P],
                                     Act.Copy)

            def emit_energy(b, vT, e_sb, mins, zsum, gz, u_sb):
                # m-outer, upper-tri; softmax(m) deferred one m-step so the
                # lower-block transposes never stall the PE on evictions.
                for m in range(CT):
                    W = C - m * P
                    ps = ps_e_pool.tile([P, CH], f32, tag="e",
                                        name=f"ps_e_{b}_{m}")
                    for p in range(NP):
                        for sub in range(2):
                            nc.tensor.matmul(
                                ps[:, :W],
                                vT[p][:, sub, m * P:(m + 1) * P],
                                vT[p][:, sub, m * P:],
                                start=(p == 0 and sub == 0),
                                stop=(p == NP - 1 and sub == 1))
                    nc.scalar.activation(e_sb[:, m, m * P:], ps[:, :W],
                                         Act.Copy)
                    if m == 0:
                        emit_softmax(b, 0, e_sb, mins, zsum, gz, u_sb)
                    else:
                        emit_lower_blocks(b, m, e_sb)
                        emit_softmax(b, m, e_sb, mins, zsum, gz, u_sb)

            def emit_ut(b, u_sb, ut_sb):
                for kt in range(CT):
                    ps_u = ps_tp.tile([P, CH], f32, tag="tp",
                                      name=f"ps_ut_{b}_{kt}")
                    for m in range(CT):
                        nc.tensor.transpose(
                            ps_u[:, m * P:(m + 1) * P],
                            u_sb[:, m, kt * P:(kt + 1) * P],
                            ident_f)
                    nc.scalar.activation(ut_sb[:, kt, :], ps_u, Act.Copy)

            def emit_mm2(b, v_sb, v_h, ut_sb, gz):
                for m in range(CT):
                    for pair in range(4):
                        out_t = out_pool.tile([P, QN], f32, tag="o",
                                              name=f"o_{b}_{m}_{pair}")
                        for sub in range(2):
                            ch = pair * 2 + sub
                            ps_o = ps_o_pool.tile([P, CH], f32, tag="o",
                                                  name=f"ps_o_{b}_{m}_{ch}")
                            for kt in range(CT):
                                nc.tensor.matmul(
                                    ps_o,
                                    ut_sb[:, kt, m * P:(m + 1) * P],
                                    v_h[kt][ch // 2][:, (ch % 2) * CH:
                                                     (ch % 2 + 1) * CH],
                                    start=(kt == 0),
                                    stop=(kt == CT - 1))
                            # out = raw * (gamma/Z_c) + x       (exact x)
                            nc.vector.scalar_tensor_tensor(
                                out_t[:, sub * CH:(sub + 1) * CH],
                                ps_o,
                                gz[:, m:m + 1],
                                v_sb[m][ch // 2][:, (ch % 2) * CH:
                                                 (ch % 2 + 1) * CH],
                                op0=Alu.mult,
                                op1=Alu.add)
                        # gpsimd SWDGE queue: output DMAs stay off the sync
                        # queue (input prefetch) and off ACT/DVE entirely
                        nc.gpsimd.dma_start(
                            o_d[b, m * P:(m + 1) * P,
                                pair * QN:(pair + 1) * QN],
                            out_t)

            # ---------------- pipelined batch loop ----------------
            v_sb, v_h = emit_load(0, cast_on_act=False)
            for b in range(BPC):
                vT = emit_transposes(b, v_h)

                mins = st_pool.tile([P, CT], f32, tag="mins", name=f"mins_{b}")
                zsum = st_pool.tile([P, CT], f32, tag="zsum", name=f"zsum_{b}")
                gz = st_pool.tile([P, CT], f32, tag="gz", name=f"gz_{b}")
                e_sb = u_pool.tile([P, CT, C], f32, tag="es", bufs=1,
                                   name=f"e_{b}")
                u_sb = u_pool.tile([P, CT, C], f32, tag="u", bufs=1,
                                   name=f"u_{b}")
                ut_sb = u_pool.tile([P, CT, C], bf16, tag="ut", name=f"ut_{b}")

                emit_energy(b, vT, e_sb, mins, zsum, gz, u_sb)
                emit_ut(b, u_sb, ut_sb)

                # prefetch next batch; casts on ACT (idle during mm2)
                if b + 1 < BPC:
                    nv_sb, nv_h = emit_load(b + 1, cast_on_act=True)

                emit_mm2(b, v_sb, v_h, ut_sb, gz)

                if b + 1 < BPC:
                    v_sb, v_h = nv_sb, nv_h

    nc.compile()
    return nc


def _get_program():
    if "nc" not in _CACHE:
        _CACHE["nc"] = _build_program()
    return _CACHE["nc"]


def kernel(x: np.ndarray, gamma: np.ndarray) -> np.ndarray:
    from concourse.bass_utils import run_bass_kernel_spmd

    assert x.shape == (B, C, 64, 64), x.shape
    x = np.ascontiguousarray(x, dtype=np.float32)
    gamma = np.ascontiguousarray(gamma, dtype=np.float32).reshape(1)

    nc = _get_program()
    xs = x.reshape(NCORES, BPC, C, N)
    in_maps = [{"x": xs[i], "gamma": gamma} for i in range(NCORES)]
    res = run_bass_kernel_spmd(nc, in_maps, list(range(NCORES)))
    out = np.empty((NCORES, BPC, C, N), dtype=np.float32)
    for i in range(NCORES):
        out[i] = res.results[i]["out"]
    return out.reshape(B, C, 64, 64)


# revision 11
# speedup vs baseline: 1.0241x; 1.0241x over previous
"""Trainium2 Bass kernel for a CAM (channel-attention) module.

Reference computation (per batch b):
    v    = x[b].reshape(C, H*W)                  # C x N
    e    = v @ v.T                               # C x C Gram matrix
    attn = softmax(rowmax(e) - e, axis=-1)       # == exp(rowmin(e)-e) / rowsum
    out  = gamma * (attn @ v) + x[b]

Sharding: data-parallel over batch B=16 across 8 NeuronCores (2 batches/core,
no cross-core communication).

Per core, per batch:
  - x[b] streamed in fp32 quarter-tiles (residual source, kept exact)
  - DVE casts to bf16 working copies (matmul operands)
  - PE-transpose bf16 128x128 blocks -> resident vT [128n, kt, 512c] (bf16),
    two kt per PSUM bank, single ACT eviction per pair
  - energy: upper-triangular blocks only (Gram symmetry); lower blocks
    reconstructed by PE-transposing finished rows; m-outer accumulation
    (one PSUM bank at a time) so softmax of block m overlaps energy of m+1
  - softmax: rowmin (DVE) + exp with accum_out rowsum (ACT); U unnormalized
  - PE-transpose U -> UT (bf16); raw[c,n] = sum_d U[c,d] v[d,n] (bf16 MMs)
  - eviction fuses normalization+gamma+residual: out = raw*(gamma/Z) + x_fp32
    (so for gamma == 0 the output is bit-exact x)
"""

import numpy as np

P = 128
C = 512
N = 4096
CT = C // P      # 4 c-tiles
NT = N // P      # 32 n-tiles
NP = NT // 2     # 16 transpose pairs
CH = 512         # matmul free-dim chunk
NCH = N // CH    # 8 n-chunks
QN = N // 4      # 1024 quarter width
B = 16
NCORES = 8
BPC = B // NCORES  # batches per core

_CACHE = {}


def _build_program():
    import concourse.bacc as bacc
    import concourse.mybir as mybir
    import concourse.tile as tile
    from concourse.masks import make_identity

    f32 = mybir.dt.float32
    bf16 = mybir.dt.bfloat16
    Alu = mybir.AluOpType
    Act = mybir.ActivationFunctionType

    nc = bacc.Bacc("TRN2", target_bir_lowering=False, debug=False)
    x_d = nc.dram_tensor("x", [BPC, C, N], f32, kind="ExternalInput").ap()
    g_d = nc.dram_tensor("gamma", [1], f32, kind="ExternalInput").ap()
    o_d = nc.dram_tensor("out", [BPC, C, N], f32, kind="ExternalOutput").ap()

    with tile.TileContext(nc) as tc:
        with (
            tc.tile_pool(name="const", bufs=1) as const_pool,
            tc.tile_pool(name="vp", bufs=1) as v_pool,
            tc.tile_pool(name="vhp", bufs=1) as vh_pool,
            tc.tile_pool(name="vtp", bufs=1) as vt_pool,
            tc.tile_pool(name="up", bufs=2) as u_pool,
            tc.tile_pool(name="stat", bufs=2) as st_pool,
            tc.tile_pool(name="outp", bufs=3) as out_pool,
            tc.tile_pool(name="pse", bufs=2, space="PSUM") as ps_e_pool,
            tc.tile_pool(name="pso", bufs=4, space="PSUM") as ps_o_pool,
            tc.tile_pool(name="pstp", bufs=2, space="PSUM") as ps_tp,
        ):
            ident_h = const_pool.tile([P, P], bf16, tag="identh")
            make_identity(nc, ident_h)
            ident_f = const_pool.tile([P, P], f32, tag="identf")
            make_identity(nc, ident_f)
            gamma_bc = const_pool.tile([P, 1], f32, tag="gamma")
            nc.sync.dma_start(gamma_bc, g_d.to_broadcast((P, 1)))

            for b in range(BPC):
                # ---- load x[b] in fp32 quarters, n-interleaved ------------
                v_sb = [[None] * 4 for _ in range(CT)]
                for q in range(4):
                    for ct in range(CT):
                        t = v_pool.tile([P, QN], f32, tag=f"v{ct}q{q}",
                                        bufs=2 if q == 0 else 1,
                                        name=f"v_{b}_{ct}_{q}")
                        nc.sync.dma_start(
                            t, x_d[b, ct * P:(ct + 1) * P, q * QN:(q + 1) * QN])
                        v_sb[ct][q] = t

                # ---- bf16 working copies (DVE casts, by quarter) ----------
                v_h = [[None, None] for _ in range(CT)]
                for q in range(4):
                    for ct in range(CT):
                        h = q // 2
                        if v_h[ct][h] is None:
                            v_h[ct][h] = vh_pool.tile(
                                [P, N // 2], bf16, tag=f"vh{ct}h{h}",
                                bufs=2 if h == 0 else 1,
                                name=f"vh_{b}_{ct}_{h}")
                        nc.vector.tensor_copy(
                            v_h[ct][h][:, (q % 2) * QN:(q % 2 + 1) * QN],
                            v_sb[ct][q])

                # ---- vT: resident bf16, built 2 kt per PSUM bank ----------
                vT = []
                for p in range(NP):
                    ps_t = ps_tp.tile([P, 2, C], bf16, tag="tp",
                                      name=f"ps_tv_{b}_{p}")
                    for sub in range(2):
                        kt = 2 * p + sub
                        h, lk = kt // 16, kt % 16
                        for ct in range(CT):
                            nc.tensor.transpose(
                                ps_t[:, sub, ct * P:(ct + 1) * P],
                                v_h[ct][h][:, lk * P:(lk + 1) * P],
                                ident_h)
                    vt_t = vt_pool.tile([P, 2, C], bf16, tag=f"vt{p}",
                                        name=f"vT_{b}_{p}")
                    nc.scalar.activation(vt_t, ps_t, Act.Copy)
                    vT.append(vt_t)

                mins = st_pool.tile([P, CT], f32, tag="mins", name=f"mins_{b}")
                zsum = st_pool.tile([P, CT], f32, tag="zsum", name=f"zsum_{b}")
                gz = st_pool.tile([P, CT], f32, tag="gz", name=f"gz_{b}")
                e_sb = u_pool.tile([P, CT, C], f32, tag="es", bufs=1,
                                   name=f"e_{b}")
                u_sb = u_pool.tile([P, CT, C], f32, tag="u", bufs=1,
                                   name=f"u_{b}")
                ut_sb = u_pool.tile([P, CT, C], bf16, tag="ut", name=f"ut_{b}")

                # ---- energy, m-outer, upper-triangular blocks only --------
                # e is symmetric: compute e[m-rows, d >= m*128]; reconstruct
                # the lower blocks by PE-transposing e[j-rows, m-cols], j<m.
                for m in range(CT):
                    W = C - m * P
                    ps = ps_e_pool.tile([P, CH], f32, tag="e",
                                        name=f"ps_e_{b}_{m}")
                    for p in range(NP):
                        for sub in range(2):
                            nc.tensor.matmul(
                                ps[:, :W],
                                vT[p][:, sub, m * P:(m + 1) * P],
                                vT[p][:, sub, m * P:],
                                start=(p == 0 and sub == 0),
                                stop=(p == NP - 1 and sub == 1))
                    nc.scalar.activation(e_sb[:, m, m * P:], ps[:, :W],
                                         Act.Copy)
                    if m > 0:
                        ps_x = ps_tp.tile([P, CH], f32, tag="tp",
                                          name=f"ps_ex_{b}_{m}")
                        for j in range(m):
                            nc.tensor.transpose(
                                ps_x[:, j * P:(j + 1) * P],
                                e_sb[:, j, m * P:(m + 1) * P],
                                ident_f)
                        nc.scalar.activation(e_sb[:, m, :m * P],
                                             ps_x[:, :m * P], Act.Copy)
                    nc.vector.tensor_reduce(
                        mins[:, m:m + 1], e_sb[:, m, :],
                        axis=mybir.AxisListType.X, op=Alu.min)
                    nc.scalar.activation(
                        u_sb[:, m, :], e_sb[:, m, :], Act.Exp,
                        bias=mins[:, m:m + 1], scale=-1.0,
                        accum_out=zsum[:, m:m + 1])
                    nc.vector.reciprocal(gz[:, m:m + 1], zsum[:, m:m + 1])
                    nc.vector.tensor_tensor(
                        gz[:, m:m + 1], gz[:, m:m + 1], gamma_bc, Alu.mult)

                # ---- UT = U.T (16 PE transposes, fp32 -> bf16) ------------
                for kt in range(CT):
                    ps_u = ps_tp.tile([P, CH], f32, tag="tp",
                                      name=f"ps_ut_{b}_{kt}")
                    for m in range(CT):
                        nc.tensor.transpose(
                            ps_u[:, m * P:(m + 1) * P],
                            u_sb[:, m, kt * P:(kt + 1) * P],
                            ident_f)
                    nc.scalar.activation(ut_sb[:, kt, :], ps_u, Act.Copy)

                # ---- raw[c,n] = sum_d U[c,d] v[d,n]; fused eviction -------
                for m in range(CT):
                    for pair in range(4):          # pairs of 512-chunks
                        out_t = out_pool.tile([P, QN], f32, tag="o",
                                              name=f"o_{b}_{m}_{pair}")
                        for sub in range(2):
                            ch = pair * 2 + sub
                            ps_o = ps_o_pool.tile([P, CH], f32, tag="o",
                                                  name=f"ps_o_{b}_{m}_{ch}")
                            for kt in range(CT):
                                nc.tensor.matmul(
                                    ps_o,
                                    ut_sb[:, kt, m * P:(m + 1) * P],
                                    v_h[kt][ch // 4][:, (ch % 4) * CH:
                                                     (ch % 4 + 1) * CH],
                                    start=(kt == 0),
                                    stop=(kt == CT - 1))
                            # out = raw * (gamma/Z_c) + x       (exact x)
                            nc.vector.scalar_tensor_tensor(
                                out_t[:, sub * CH:(sub + 1) * CH],
                                ps_o,
                                gz[:, m:m + 1],
                                v_sb[m][ch // 2][:, (ch % 2) * CH:
                                                 (ch % 2 + 1) * CH],
                                op0=Alu.mult,
                                op1=Alu.add)
                        nc.sync.dma_start(
                            o_d[b, m * P:(m + 1) * P,
                                pair * QN:(pair + 1) * QN],
                            out_t)

    nc.compile()
    return nc


def _get_program():
    if "nc" not in _CACHE:
        _CACHE["nc"] = _build_program()
    return _CACHE["nc"]


def kernel(x: np.ndarray, gamma: np.ndarray) -> np.ndarray:
    from concourse.bass_utils import run_bass_kernel_spmd

    assert x.shape == (B, C, 64, 64), x.shape
    x = np.ascontiguousarray(x, dtype=np.float32)
    gamma = np.ascontiguousarray(gamma, dtype=np.float32).reshape(1)

    nc = _get_program()
    xs = x.reshape(NCORES, BPC, C, N)
    in_maps = [{"x": xs[i], "gamma": gamma} for i in range(NCORES)]
    res = run_bass_kernel_spmd(nc, in_maps, list(range(NCORES)))
    out = np.empty((NCORES, BPC, C, N), dtype=np.float32)
    for i in range(NCORES):
        out[i] = res.results[i]["out"]
    return out.reshape(B, C, 64, 64)


# revision 12
# speedup vs baseline: 1.0905x; 1.0649x over previous
"""Trainium2 Bass kernel for a CAM (channel-attention) module.

Reference computation (per batch b):
    v    = x[b].reshape(C, H*W)                  # C x N
    e    = v @ v.T                               # C x C Gram matrix
    attn = softmax(rowmax(e) - e, axis=-1)       # == exp(rowmin(e)-e) / rowsum
    out  = gamma * (attn @ v) + x[b]

Sharding: data-parallel over batch B=16 across 8 NeuronCores (2 batches/core,
no cross-core communication).

Per core, per batch:
  - x[b] streamed in fp32 quarter-tiles (residual source, kept exact)
  - DVE casts to bf16 working copies (matmul operands)
  - PE-transpose bf16 128x128 blocks -> resident vT [128n, kt, 512c] (bf16),
    two kt per PSUM bank, single ACT eviction per pair
  - energy: upper-triangular blocks only (Gram symmetry); lower blocks
    reconstructed by PE-transposing finished rows; m-outer accumulation
    (one PSUM bank at a time) so softmax of block m overlaps energy of m+1
  - softmax: rowmin (DVE) + exp with accum_out rowsum (ACT); U unnormalized
  - PE-transpose U -> UT (bf16); raw[c,n] = sum_d U[c,d] v[d,n] (bf16 MMs)
  - eviction fuses normalization+gamma+residual: out = raw*(gamma/Z) + x_fp32
    (so for gamma == 0 the output is bit-exact x)
"""

import numpy as np

P = 128
C = 512
N = 4096
CT = C // P      # 4 c-tiles
NT = N // P      # 32 n-tiles
NP = NT // 2     # 16 transpose pairs
CH = 512         # matmul free-dim chunk
NCH = N // CH    # 8 n-chunks
QN = N // 4      # 1024 quarter width
B = 16
NCORES = 8
BPC = B // NCORES  # batches per core

_CACHE = {}


def _build_program():
    import concourse.bacc as bacc
    import concourse.mybir as mybir
    import concourse.tile as tile
    from concourse.masks import make_identity

    f32 = mybir.dt.float32
    bf16 = mybir.dt.bfloat16
    Alu = mybir.AluOpType
    Act = mybir.ActivationFunctionType

    nc = bacc.Bacc("TRN2", target_bir_lowering=False, debug=False)
    x_d = nc.dram_tensor("x", [BPC, C, N], f32, kind="ExternalInput").ap()
    g_d = nc.dram_tensor("gamma", [1], f32, kind="ExternalInput").ap()
    o_d = nc.dram_tensor("out", [BPC, C, N], f32, kind="ExternalOutput").ap()

    with tile.TileContext(nc) as tc:
        with (
            tc.tile_pool(name="const", bufs=1) as const_pool,
            tc.tile_pool(name="vp", bufs=1) as v_pool,
            tc.tile_pool(name="vhp", bufs=1) as vh_pool,
            tc.tile_pool(name="vtp", bufs=1) as vt_pool,
            tc.tile_pool(name="up", bufs=2) as u_pool,
            tc.tile_pool(name="stat", bufs=2) as st_pool,
            tc.tile_pool(name="outp", bufs=4) as out_pool,
            tc.tile_pool(name="pse", bufs=2, space="PSUM") as ps_e_pool,
            tc.tile_pool(name="pso", bufs=4, space="PSUM") as ps_o_pool,
            tc.tile_pool(name="pstp", bufs=2, space="PSUM") as ps_tp,
        ):
            ident_h = const_pool.tile([P, P], bf16, tag="identh")
            make_identity(nc, ident_h)
            ident_f = const_pool.tile([P, P], f32, tag="identf")
            make_identity(nc, ident_f)
            gamma_bc = const_pool.tile([P, 1], f32, tag="gamma")
            nc.sync.dma_start(gamma_bc, g_d.to_broadcast((P, 1)))

            for b in range(BPC):
                # ---- load x[b] in fp32 quarters, n-interleaved ------------
                v_sb = [[None] * 4 for _ in range(CT)]
                for q in range(4):
                    for ct in range(CT):
                        t = v_pool.tile([P, QN], f32, tag=f"v{ct}q{q}",
                                        bufs=2 if q == 0 else 1,
                                        name=f"v_{b}_{ct}_{q}")
                        nc.sync.dma_start(
                            t, x_d[b, ct * P:(ct + 1) * P, q * QN:(q + 1) * QN])
                        v_sb[ct][q] = t

                # ---- bf16 working copies (DVE casts, quarter tiles) -------
                v_h = [[None] * 4 for _ in range(CT)]
                for q in range(4):
                    for ct in range(CT):
                        th = vh_pool.tile([P, QN], bf16, tag=f"vh{ct}q{q}",
                                          bufs=2 if q == 0 else 1,
                                          name=f"vh_{b}_{ct}_{q}")
                        nc.vector.tensor_copy(th, v_sb[ct][q])
                        v_h[ct][q] = th

                # ---- vT: resident bf16, built 2 kt per PSUM bank ----------
                vT = []
                for p in range(NP):
                    ps_t = ps_tp.tile([P, 2, C], bf16, tag="tp",
                                      name=f"ps_tv_{b}_{p}")
                    for sub in range(2):
                        kt = 2 * p + sub
                        q, lk = kt // 8, kt % 8
                        for ct in range(CT):
                            nc.tensor.transpose(
                                ps_t[:, sub, ct * P:(ct + 1) * P],
                                v_h[ct][q][:, lk * P:(lk + 1) * P],
                                ident_h)
                    vt_t = vt_pool.tile([P, 2, C], bf16, tag=f"vt{p}",
                                        name=f"vT_{b}_{p}")
                    nc.scalar.activation(vt_t, ps_t, Act.Copy)
                    vT.append(vt_t)

                mins = st_pool.tile([P, CT], f32, tag="mins", name=f"mins_{b}")
                zsum = st_pool.tile([P, CT], f32, tag="zsum", name=f"zsum_{b}")
                gz = st_pool.tile([P, CT], f32, tag="gz", name=f"gz_{b}")
                e_sb = u_pool.tile([P, CT, C], f32, tag="es", bufs=1,
                                   name=f"e_{b}")
                u_sb = u_pool.tile([P, CT, C], f32, tag="u", bufs=1,
                                   name=f"u_{b}")
                ut_sb = u_pool.tile([P, CT, C], bf16, tag="ut", name=f"ut_{b}")

                # ---- energy, m-outer, upper-triangular blocks only --------
                # e is symmetric: compute e[m-rows, d >= m*128]; reconstruct
                # the lower blocks by PE-transposing e[j-rows, m-cols], j<m.
                for m in range(CT):
                    W = C - m * P
                    ps = ps_e_pool.tile([P, CH], f32, tag="e",
                                        name=f"ps_e_{b}_{m}")
                    for p in range(NP):
                        for sub in range(2):
                            nc.tensor.matmul(
                                ps[:, :W],
                                vT[p][:, sub, m * P:(m + 1) * P],
                                vT[p][:, sub, m * P:],
                                start=(p == 0 and sub == 0),
                                stop=(p == NP - 1 and sub == 1))
                    nc.scalar.activation(e_sb[:, m, m * P:], ps[:, :W],
                                         Act.Copy)
                    if m > 0:
                        ps_x = ps_tp.tile([P, CH], f32, tag="tp",
                                          name=f"ps_ex_{b}_{m}")
                        for j in range(m):
                            nc.tensor.transpose(
                                ps_x[:, j * P:(j + 1) * P],
                                e_sb[:, j, m * P:(m + 1) * P],
                                ident_f)
                        nc.scalar.activation(e_sb[:, m, :m * P],
                                             ps_x[:, :m * P], Act.Copy)
                    nc.vector.tensor_reduce(
                        mins[:, m:m + 1], e_sb[:, m, :],
                        axis=mybir.AxisListType.X, op=Alu.min)
                    nc.scalar.activation(
                        u_sb[:, m, :], e_sb[:, m, :], Act.Exp,
                        bias=mins[:, m:m + 1], scale=-1.0,
                        accum_out=zsum[:, m:m + 1])
                    nc.vector.reciprocal(gz[:, m:m + 1], zsum[:, m:m + 1])
                    nc.vector.tensor_tensor(
                        gz[:, m:m + 1], gz[:, m:m + 1], gamma_bc, Alu.mult)

                # ---- UT = U.T (16 PE transposes, fp32 -> bf16) ------------
                for kt in range(CT):
                    ps_u = ps_tp.tile([P, CH], f32, tag="tp",
                                      name=f"ps_ut_{b}_{kt}")
                    for m in range(CT):
                        nc.tensor.transpose(
                            ps_u[:, m * P:(m + 1) * P],
                            u_sb[:, m, kt * P:(kt + 1) * P],
                            ident_f)
                    nc.scalar.activation(ut_sb[:, kt, :], ps_u, Act.Copy)

                # ---- raw[c,n] = sum_d U[c,d] v[d,n]; fused eviction -------
                for m in range(CT):
                    for pair in range(4):          # pairs of 512-chunks
                        out_t = out_pool.tile([P, QN], f32, tag="o",
                                              name=f"o_{b}_{m}_{pair}")
                        for sub in range(2):
                            ch = pair * 2 + sub
                            ps_o = ps_o_pool.tile([P, CH], f32, tag="o",
                                                  name=f"ps_o_{b}_{m}_{ch}")
                            for kt in range(CT):
                                nc.tensor.matmul(
                                    ps_o,
                                    ut_sb[:, kt, m * P:(m + 1) * P],
                                    v_h[kt][ch // 2][:, (ch % 2) * CH:
                                                     (ch % 2 + 1) * CH],
                                    start=(kt == 0),
                                    stop=(kt == CT - 1))
                            # out = raw * (gamma/Z_c) + x       (exact x)
                            nc.vector.scalar_tensor_tensor(
                                out_t[:, sub * CH:(sub + 1) * CH],
                                ps_o,
                                gz[:, m:m + 1],
                                v_sb[m][ch // 2][:, (ch % 2) * CH:
                                                 (ch % 2 + 1) * CH],
                                op0=Alu.mult,
                                op1=Alu.add)
                        nc.sync.dma_start(
                            o_d[b, m * P:(m + 1) * P,
                                pair * QN:(pair + 1) * QN],
                            out_t)

    nc.compile()
    return nc


def _get_program():
    if "nc" not in _CACHE:
        _CACHE["nc"] = _build_program()
    return _CACHE["nc"]


def kernel(x: np.ndarray, gamma: np.ndarray) -> np.ndarray:
    from concourse.bass_utils import run_bass_kernel_spmd

    assert x.shape == (B, C, 64, 64), x.shape
    x = np.ascontiguousarray(x, dtype=np.float32)
    gamma = np.ascontiguousarray(gamma, dtype=np.float32).reshape(1)

    nc = _get_program()
    xs = x.reshape(NCORES, BPC, C, N)
    in_maps = [{"x": xs[i], "gamma": gamma} for i in range(NCORES)]
    res = run_bass_kernel_spmd(nc, in_maps, list(range(NCORES)))
    out = np.empty((NCORES, BPC, C, N), dtype=np.float32)
    for i in range(NCORES):
        out[i] = res.results[i]["out"]
    return out.reshape(B, C, 64, 64)
